# revision 2
# baseline (speedup 1.0000x reference)
"""CausalPointNetEncoder v4 — software-pipelined, 1024-col matmuls.

v3 -> v4:
 - All matmuls use 1024-col moving operands (fp16 max) -> half the
   MATMUL+LDWEIGHTS instruction count.
 - P01 software pipeline with LAG: x0 staged in bigA, p0 staged in bigC
   (overwritten by raw1 chunk-by-chunk). PE queue: mm0(c), mm1(c-LAG)
   back-to-back so the PE never stalls on the scan chain (keeps the HAM
   activity window hot -> 2.4 GHz).
 - P2/P3 pipelined the same way (apply prefetched AHEAD of the
   stats/evac ops of older chunks; psum tiles alternate psA/psB pools).
 - P4: packed output via accumulate trick at 1024 cols; contiguous
   [128, R/2] output layout (host unpacks); DMA on gpsimd queue.
 - Layer-0 stats exact on host (input gram); split-AllReduce for
   layers 1-3.

Phase engine budgets (us, 2048-col chunks x16):
  P01: ACT x0 31.5 + evac1 36 + sq-share | DVE scan0 73 + sq-share | PE ~60@hot
  P2:  ACT x1 35.6 + sq2 44.7 | DVE scan2 73 | PE 20
  P3:  ACT evac3 36 | DVE p2 12 + sq3 37 | PE 20
  P4:  ACT bias 18 | PE 20 | DMA 12
"""

import numpy as np

import concourse.bass as bass
import concourse.mybir as mybir
from concourse.tile import TileContext
from concourse.bass_utils import run_bass_kernel_spmd

FP16 = np.float16

B, A, T, C, H, O = 16, 64, 256, 32, 128, 64
N_CORES = 8
BA = B * A
P_CORE = BA // N_CORES
R = P_CORE * T
N_TOTAL = BA * T
EPS = 1e-5

CH = 2048
NCH = R // CH
PS = 1024
NEG = -60000.0
LAG = 3          # P01 software-pipeline depth (chunks)
LAG2 = 2         # P2/P3 pipeline depth

F32 = mybir.dt.float32
BF = mybir.dt.float16
ALU = mybir.AluOpType
AFT = mybir.ActivationFunctionType


def _split_multi_waits(nc):
    cnt = 0
    for f in nc.m.functions:
        for bb in f.blocks:
            il = bb.instructions
            if not any(i.sync_info and len(i.sync_info.on_wait) > 1 for i in il):
                continue
            new = []
            for inst in il:
                si = inst.sync_info
                waits = list(si.on_wait) if si else []
                if len(waits) > 1:
                    for w in waits[:-1]:
                        nop = mybir.InstNoOp(name=f"I-wsplit-{cnt}", ins=[], outs=[])
                        cnt += 1
                        nop.engine = inst.engine
                        nop.sync_info = mybir.SyncInfo(on_wait=[w], on_update=[])
                        new.append(nop)
                    inst.sync_info = mybir.SyncInfo(
                        on_wait=[waits[-1]], on_update=list(si.on_update)
                    )
                new.append(inst)
            bb.instructions = new
    return cnt


def build_nc():
    nc = bass.Bass()

    xin = nc.declare_dram_parameter("xfm", [C, R], BF, isOutput=False)
    w0 = nc.declare_dram_parameter("w0", [C, H], BF, isOutput=False)
    w1t = nc.declare_dram_parameter("w1t", [H, H], BF, isOutput=False)
    w1b = nc.declare_dram_parameter("w1b", [H, H], BF, isOutput=False)
    w2 = nc.declare_dram_parameter("w2", [H, H], BF, isOutput=False)
    w3 = nc.declare_dram_parameter("w3", [H, H], BF, isOutput=False)
    w4l = nc.declare_dram_parameter("w4l", [H, H], BF, isOutput=False)
    w4r = nc.declare_dram_parameter("w4r", [H, H], BF, isOutput=False)
    c10 = nc.declare_dram_parameter("c10", [H, 1], F32, isOutput=False)
    gabs = nc.declare_dram_parameter("gabs", [H, 4], F32, isOutput=False)
    bes = nc.declare_dram_parameter("bes", [H, 4], F32, isOutput=False)
    b4d = nc.declare_dram_parameter("b4d", [H, 1], F32, isOutput=False)
    # packed output: partitions 0-63 = even 1024-col groups, 64-127 = odd
    yout = nc.declare_dram_parameter("out", [H, R // 2], BF, isOutput=True)

    partsA = [None] + [nc.dram_tensor(f"partA{k}", [H, 2], F32) for k in (1, 2, 3)]
    redsA = [None] + [nc.dram_tensor(f"redA{k}", [H, 2], F32) for k in (1, 2, 3)]
    partsB = [None] + [nc.dram_tensor(f"partB{k}", [H, 2], F32) for k in (1, 2, 3)]
    redsB = [None] + [nc.dram_tensor(f"redB{k}", [H, 2], F32) for k in (1, 2, 3)]
    warm_p = nc.dram_tensor("warm_p", [H, 2], F32)
    warm_r = nc.dram_tensor("warm_r", [H, 2], F32)

    with TileContext(nc) as tc:
        with (
            tc.tile_pool(name="sing", bufs=1) as sing,
            tc.tile_pool(name="big", bufs=1) as big,
            tc.tile_pool(name="inp", bufs=4) as inp,
            tc.tile_pool(name="roll", bufs=LAG2 + 2) as roll,
            tc.tile_pool(name="scr", bufs=2) as scrp,
            tc.tile_pool(name="ot", bufs=3) as otp,
            tc.tile_pool(name="stat", bufs=1) as stat,
            tc.tile_pool(name="psA", bufs=2, space="PSUM") as psA,
            tc.tile_pool(name="psB", bufs=2, space="PSUM") as psB,
        ):
            # ---- critical weights on sync queue ----
            w0_sb = sing.tile([C, H], BF, tag="w0")
            nc.sync.dma_start(out=w0_sb, in_=w0[:])
            c10_sb = sing.tile([H, 1], F32, tag="c10")
            nc.sync.dma_start(out=c10_sb, in_=c10[:])
            w1t_sb = sing.tile([H, H], BF, tag="w1t")
            nc.sync.dma_start(out=w1t_sb, in_=w1t[:])
            w1b_sb = sing.tile([H, H], BF, tag="w1b")
            nc.sync.dma_start(out=w1b_sb, in_=w1b[:])
            # ---- non-critical weights on gpsimd queue ----
            w2_sb = sing.tile([H, H], BF, tag="w2")
            nc.gpsimd.dma_start(out=w2_sb, in_=w2[:])
            w3_sb = sing.tile([H, H], BF, tag="w3")
            nc.gpsimd.dma_start(out=w3_sb, in_=w3[:])
            w4l_sb = sing.tile([H, H], BF, tag="w4l")
            nc.gpsimd.dma_start(out=w4l_sb, in_=w4l[:])
            w4r_sb = sing.tile([H, H], BF, tag="w4r")
            nc.gpsimd.dma_start(out=w4r_sb, in_=w4r[:])
            gabs_sb = sing.tile([H, 4], F32, tag="gabs")
            nc.gpsimd.dma_start(out=gabs_sb, in_=gabs[:])
            bes_sb = sing.tile([H, 4], F32, tag="bes")
            nc.gpsimd.dma_start(out=bes_sb, in_=bes[:])
            b4d_sb = sing.tile([H, 1], F32, tag="b4d")
            nc.gpsimd.dma_start(out=b4d_sb, in_=b4d[:])

            w2_s = sing.tile([H, H], BF, tag="w2_s")
            w3_s = sing.tile([H, H], BF, tag="w3_s")
            w4l_s = sing.tile([H, H], BF, tag="w4l_s")
            w4r_s = sing.tile([H, H], BF, tag="w4r_s")
            w2_sf = sing.tile([H, H], F32, tag="w2_sf")

            mask01 = sing.tile([H, CH], BF, tag="mask01")
            nc.vector.memset(mask01, 1.0)
            nc.vector.memset(
                mask01.rearrange("p (n t) -> p n t", t=T)[:, :, 0:1], 0.0
            )
            maskneg = sing.tile([H, PS], BF, tag="maskneg")
            nc.vector.memset(maskneg, 0.0)
            nc.vector.memset(
                maskneg.rearrange("p (n t) -> p n t", t=T)[:, :, 0:1], NEG
            )

            wtile = stat.tile([H, 2], F32, tag="wtile")
            nc.vector.memset(wtile, 0.0)
            nc.sync.dma_start(out=warm_p[:], in_=wtile)
            nc.gpsimd.collective_compute(
                "AllReduce", ALU.add,
                replica_groups=[list(range(N_CORES))],
                ins=[warm_p[:]], outs=[warm_r[:]],
            )

            # bigA: x0 (P01) -> cmax2 (P2) -> read by P3
            # bigC: p0 -> raw1 (P01, chunk-wise overwrite) -> raw3 (P3) -> P4
            bigA = big.tile([H, R], BF, tag="bigA")
            bigC = big.tile([H, R], BF, tag="bigC")

            accS1 = stat.tile([H, 2 * NCH], F32, tag="accS1")
            accQ1 = stat.tile([H, 2 * NCH], F32, tag="accQ1")
            accX1 = stat.tile([H, NCH], F32, tag="accX1")
            accQ2 = stat.tile([H, 2 * NCH], F32, tag="accQ2")
            accS3 = stat.tile([H, 2 * NCH], F32, tag="accS3")
            accQ3 = stat.tile([H, 2 * NCH], F32, tag="accQ3")
            for t_ in (accS1, accQ1, accX1, accQ2, accS3, accQ3):
                nc.vector.memset(t_, 0.0)

            c1 = [None] + [stat.tile([H, 1], F32, name=f"c1_{k}", tag=f"c1_{k}") for k in (1, 2, 3)]
            svec = [None] + [stat.tile([H, 1], F32, name=f"s_{k}", tag=f"s_{k}") for k in (1, 2, 3)]
            tmp1 = stat.tile([H, 1], F32, tag="tmp1")
            tmp2 = stat.tile([H, 1], F32, tag="tmp2")
            tmp3 = stat.tile([H, 1], F32, tag="tmp3")
            musq = stat.tile([H, 1], F32, tag="musq")
            std = stat.tile([H, 1], F32, tag="std")
            rstd = stat.tile([H, 1], F32, tag="rstd")
            recs = stat.tile([H, 1], F32, tag="recs")
            gpartA = stat.tile([H, 2], F32, tag="gpartA")
            gpartB = stat.tile([H, 2], F32, tag="gpartB")
            gstat = [None] + [stat.tile([H, 2], F32, name=f"gstat{k}", tag=f"gstat{k}") for k in (1, 2, 3)]
            gstatA = [None] + [stat.tile([H, 2], F32, name=f"gstatA{k}", tag=f"gstatA{k}") for k in (1, 2, 3)]
            gstatB = [None] + [stat.tile([H, 2], F32, name=f"gstatB{k}", tag=f"gstatB{k}") for k in (1, 2, 3)]
            eps_sb = stat.tile([H, 1], F32, tag="eps")
            nc.vector.memset(eps_sb, EPS)

            def ar_partial(k, which, sl_s, sl_q, accS_, accQ_):
                gp = gpartA if which == "A" else gpartB
                pt_ = (partsA if which == "A" else partsB)[k]
                rd_ = (redsA if which == "A" else redsB)[k]
                gs_ = (gstatA if which == "A" else gstatB)[k]
                nc.vector.tensor_reduce(
                    gp[:, 0:1], accS_[:, sl_s[0] : sl_s[1]],
                    mybir.AxisListType.X, ALU.add,
                )
                nc.vector.tensor_reduce(
                    gp[:, 1:2], accQ_[:, sl_q[0] : sl_q[1]],
                    mybir.AxisListType.X, ALU.add,
                )
                nc.sync.dma_start(out=pt_[:], in_=gp)
                nc.gpsimd.collective_compute(
                    "AllReduce", ALU.add,
                    replica_groups=[list(range(N_CORES))],
                    ins=[pt_[:]], outs=[rd_[:]],
                )
                nc.sync.dma_start(out=gs_, in_=rd_[:])

            def barrier(k, trickW, w_scale_jobs):
                g = gstat[k]
                nc.vector.tensor_tensor(g, gstatA[k], gstatB[k], ALU.add)
                sumv = g[:, 0:1]
                if trickW is not None:
                    ps1 = psA.tile([H, PS], F32, tag="mmps")
                    nc.vector.tensor_copy(tmp3, sumv)
                    nc.tensor.matmul(ps1[:, 0:1], lhsT=trickW, rhs=tmp3,
                                     start=True, stop=True)
                    nc.vector.tensor_copy(tmp1, ps1[:, 0:1])
                    sumv = tmp1
                mu = tmp3
                nc.vector.tensor_scalar(mu, sumv, 1.0 / N_TOTAL, None, ALU.mult)
                nc.vector.tensor_scalar(tmp2, g[:, 1:2], 1.0 / N_TOTAL, None, ALU.mult)
                var = tmp2
                nc.vector.tensor_tensor(musq, mu, mu, ALU.mult)
                nc.vector.tensor_tensor(var, var, musq, ALU.subtract)
                nc.scalar.activation(std, var, AFT.Sqrt, bias=eps_sb, scale=1.0)
                nc.vector.reciprocal(rstd, std)
                nc.vector.tensor_tensor(svec[k], rstd, gabs_sb[:, k : k + 1], ALU.mult)
                nc.vector.tensor_scalar(svec[k], svec[k], 1e-20, None, ALU.max)
                nc.vector.reciprocal(recs, svec[k])
                nc.vector.tensor_tensor(c1[k], bes_sb[:, k : k + 1], recs, ALU.mult)
                nc.vector.tensor_tensor(c1[k], c1[k], mu, ALU.subtract)
                for wdst, wsrc in w_scale_jobs:
                    nc.vector.tensor_scalar(wdst, wsrc, svec[k], None, ALU.mult)

            # ====== P01 (software-pipelined, LAG chunks) ==================
            # stage A(ci): dma; mm0 -> psA [H,1024]x2; x0=Relu+c10 -> bigA;
            #              scan0(x0) -> bigC (p0)
            # stage B(cj=ci-LAG): mm1 (w1t*x0 + w1b*p0) -> psB [H,1024]x2;
            #              evac1+acc -> bigC (raw1 overwrites p0);
            #              sq1 stt (psB x raw1-sbuf) -> accQ1
            def p01_stageA(ci):
                cs = ci * CH
                xt = inp.tile([C, CH], BF, tag="xin")
                nc.sync.dma_start(out=xt, in_=xin[:, cs : cs + CH])
                for h in range(2):
                    pa = psA.tile([H, PS], F32, tag="mmps")
                    for q in range(2):
                        nc.tensor.matmul(
                            pa[:, q * 512 : (q + 1) * 512], lhsT=w0_sb,
                            rhs=xt[:, h * PS + q * 512 : h * PS + (q + 1) * 512],
                            start=True, stop=True,
                        )
                    nc.scalar.activation(
                        bigA[:, cs + h * PS : cs + (h + 1) * PS], pa,
                        AFT.Relu, bias=c10_sb, scale=1.0,
                    )
                nc.vector.tensor_tensor_scan(
                    bigC[:, cs : cs + CH], mask01, bigA[:, cs : cs + CH],
                    0.0, ALU.mult, ALU.max,
                )

            def p01_stageB(cj):
                cs = cj * CH
                for h in range(2):
                    sl0 = slice(cs + h * PS, cs + h * PS + 512)
                    sl1 = slice(cs + h * PS + 512, cs + (h + 1) * PS)
                    sl = slice(cs + h * PS, cs + (h + 1) * PS)
                    pb = psB.tile([H, PS], F32, tag="mmps")
                    nc.tensor.matmul(
                        pb[:, 0:512], lhsT=w1t_sb, rhs=bigA[:, sl0],
                        start=True, stop=False,
                    )
                    nc.tensor.matmul(
                        pb[:, 512:PS], lhsT=w1t_sb, rhs=bigA[:, sl1],
                        start=True, stop=False,
                    )
                    nc.tensor.matmul(
                        pb[:, 0:512], lhsT=w1b_sb, rhs=bigC[:, sl0],
                        start=False, stop=True,
                    )
                    nc.tensor.matmul(
                        pb[:, 512:PS], lhsT=w1b_sb, rhs=bigC[:, sl1],
                        start=False, stop=True,
                    )
                    col = 2 * cj + h
                    nc.scalar.activation(
                        bigC[:, sl], pb, AFT.Copy,
                        accum_out=accS1[:, col : col + 1],
                    )
                    scr = scrp.tile([H, PS], BF, tag="scr")
                    if h == 0:
                        nc.vector.scalar_tensor_tensor(
                            scr, pb, 1.0, bigC[:, sl], ALU.mult, ALU.mult,
                            accum_out=accQ1[:, col : col + 1],
                        )
                    else:
                        nc.scalar.activation(
                            scr, pb, AFT.Square,
                            accum_out=accQ1[:, col : col + 1],
                        )

            for ci in range(NCH + LAG):
                if ci < NCH:
                    p01_stageA(ci)
                if ci >= LAG:
                    p01_stageB(ci - LAG)
                if ci - LAG == NCH - 3:
                    ar_partial(1, "A", (0, 2 * (NCH - 2)), (0, 2 * (NCH - 2)),
                               accS1, accQ1)
            ar_partial(1, "B", (2 * (NCH - 2), 2 * NCH), (2 * (NCH - 2), 2 * NCH),
                       accS1, accQ1)
            barrier(1, None, [(w2_s, w2_sb)])
            nc.vector.tensor_copy(w2_sf, w2_s)

            # ====== P2 (pipelined LAG2): x1 ahead; mm2/scan2/sq2 behind ===
            # x1 roll tiles; mm2 alternates psA/psB pools
            x1_tiles = {}

            def p2_stageA(ci):
                cs = ci * CH
                x1 = roll.tile([H, CH], BF, tag="xroll")
                nc.scalar.activation(
                    x1, bigC[:, cs : cs + CH],
                    AFT.Relu, bias=c1[1], scale=1.0,
                    accum_out=accX1[:, ci : ci + 1],
                )
                x1_tiles[ci] = x1

            def p2_stageB(cj):
                cs = cj * CH
                x1 = x1_tiles.pop(cj)
                for h in range(2):
                    pool = psA if h == 0 else psB
                    pa = pool.tile([H, PS], F32, tag="mmps")
                    for q in range(2):
                        nc.tensor.matmul(
                            pa[:, q * 512 : (q + 1) * 512], lhsT=w2_s,
                            rhs=x1[:, h * PS + q * 512 : h * PS + (q + 1) * 512],
                            start=True, stop=True,
                        )
                    col = 2 * cj + h
                    nc.vector.tensor_tensor_scan(
                        bigA[:, cs + h * PS : cs + (h + 1) * PS],
                        maskneg, pa, NEG, ALU.add, ALU.max,
                    )
                    scr2 = scrp.tile([H, PS], BF, tag="scr")
                    nc.scalar.activation(
                        scr2, pa, AFT.Square,
                        accum_out=accQ2[:, col : col + 1],
                    )

            for ci in range(NCH + LAG2):
                if ci < NCH:
                    p2_stageA(ci)
                if ci >= LAG2:
                    p2_stageB(ci - LAG2)
                if ci - LAG2 == NCH - 3:
                    ar_partial(2, "A", (0, NCH - 2), (0, 2 * (NCH - 2)),
                               accX1, accQ2)
            ar_partial(2, "B", (NCH - 2, NCH), (2 * (NCH - 2), 2 * NCH),
                       accX1, accQ2)
            barrier(2, w2_sf, [(w3_s, w3_sb)])

            # ====== P3 (pipelined LAG2): p2 ahead; mm3/evac3/sq3 behind ===
            p2_tiles = {}

            def p3_stageA(ci):
                cs = ci * CH
                p2 = roll.tile([H, CH], BF, tag="xroll")
                nc.vector.tensor_scalar(
                    p2, bigA[:, cs : cs + CH], c1[2], 0.0, ALU.add, ALU.max
                )
                p2_tiles[ci] = p2

            def p3_stageB(cj):
                cs = cj * CH
                p2 = p2_tiles.pop(cj)
                for h in range(2):
                    pool = psA if h == 0 else psB
                    pb = pool.tile([H, PS], F32, tag="mmps")
                    for q in range(2):
                        nc.tensor.matmul(
                            pb[:, q * 512 : (q + 1) * 512], lhsT=w3_s,
                            rhs=p2[:, h * PS + q * 512 : h * PS + (q + 1) * 512],
                            start=True, stop=True,
                        )
                    sl = slice(cs + h * PS, cs + (h + 1) * PS)
                    col = 2 * cj + h
                    nc.scalar.activation(
                        bigC[:, sl], pb, AFT.Copy,
                        accum_out=accS3[:, col : col + 1],
                    )
                    scr = scrp.tile([H, PS], BF, tag="scr")
                    nc.vector.scalar_tensor_tensor(
                        scr, pb, 1.0, bigC[:, sl], ALU.mult, ALU.mult,
                        accum_out=accQ3[:, col : col + 1],
                    )

            for ci in range(NCH + LAG2):
                if ci < NCH:
                    p3_stageA(ci)
                if ci >= LAG2:
                    p3_stageB(ci - LAG2)
                if ci - LAG2 == NCH - 3:
                    ar_partial(3, "A", (0, 2 * (NCH - 2)), (0, 2 * (NCH - 2)),
                               accS3, accQ3)
            ar_partial(3, "B", (2 * (NCH - 2), 2 * NCH), (2 * (NCH - 2), 2 * NCH),
                       accS3, accQ3)
            barrier(3, None, [(w4l_s, w4l_sb), (w4r_s, w4r_sb)])

            # ====== P4: x3; packed mm4 (1024-col accumulate trick) ========
            for ci in range(NCH):
                cs = ci * CH
                x3 = roll.tile([H, CH], BF, tag="xroll")
                nc.vector.tensor_scalar(
                    x3, bigC[:, cs : cs + CH], c1[3], 0.0, ALU.add, ALU.max
                )
                pool = psA if ci % 2 == 0 else psB
                pa = pool.tile([H, PS], F32, tag="mmps")
                nc.tensor.matmul(
                    pa[:, 0:512], lhsT=w4l_s, rhs=x3[:, 0:512],
                    start=True, stop=False,
                )
                nc.tensor.matmul(
                    pa[:, 512:PS], lhsT=w4l_s, rhs=x3[:, 512:PS],
                    start=True, stop=False,
                )
                nc.tensor.matmul(
                    pa[:, 0:512], lhsT=w4r_s, rhs=x3[:, PS : PS + 512],
                    start=False, stop=True,
                )
                nc.tensor.matmul(
                    pa[:, 512:PS], lhsT=w4r_s, rhs=x3[:, PS + 512 : CH],
                    start=False, stop=True,
                )
                ot = otp.tile([H, PS], BF, tag="ot")
                nc.scalar.activation(ot, pa, AFT.Identity, bias=b4d_sb, scale=1.0)
                nc.gpsimd.dma_start(
                    out=yout[:, ci * PS : (ci + 1) * PS], in_=ot
                )

    _split_multi_waits(nc)
    return nc


_NC_CACHE = None


def kernel(**inputs):
    global _NC_CACHE
    pl = np.asarray(inputs["polylines"], np.float32).reshape(BA, T, C)
    W0 = np.asarray(inputs["W0"], np.float32)
    W1 = np.asarray(inputs["W1"], np.float32)
    W2 = np.asarray(inputs["W2"], np.float32)
    W3 = np.asarray(inputs["W3"], np.float32)
    W4 = np.asarray(inputs["W4"], np.float32)
    b4v = np.asarray(inputs["b4"], np.float32)
    g = [np.asarray(inputs[f"g{k}"], np.float32) for k in range(4)]
    be = [np.asarray(inputs[f"be{k}"], np.float32) for k in range(4)]

    sg = [np.where(gk < 0, -1.0, 1.0).astype(np.float32) for gk in g]

    # ---- host layer-0 stats (exact, fp64, on fp16-rounded input) ----
    x16 = pl.reshape(N_TOTAL, C).astype(FP16)
    W0f16 = (W0 * sg[0][None, :]).astype(FP16)
    x64 = x16.astype(np.float64)
    W064 = W0f16.astype(np.float64)
    sum_x = x64.sum(0)
    Gin = x64.T @ x64
    sum0 = sum_x @ W064
    sumsq0 = np.einsum("if,ij,jf->f", W064, Gin, W064)
    mu0 = sum0 / N_TOTAL
    var0 = sumsq0 / N_TOTAL - mu0 * mu0
    s0 = np.abs(g[0]).astype(np.float64) / np.sqrt(var0 + EPS)
    s0 = np.maximum(s0, 1e-20)
    c1_0 = (be[0].astype(np.float64) / s0 - mu0).astype(np.float32)

    W1f = W1 * sg[1][None, :]
    W1ts = (s0[:, None].astype(np.float32) * W1f[:H]).astype(FP16)
    W1bs = (s0[:, None].astype(np.float32) * W1f[H:]).astype(FP16)
    W2f = (W2 * sg[2][None, :]).astype(FP16)
    W3f = (W3 * sg[3][None, :]).astype(FP16)
    W4l = np.zeros((H, H), np.float32)
    W4r = np.zeros((H, H), np.float32)
    W4l[:, :O] = W4
    W4r[:, O:] = W4
    b4dup = np.concatenate([b4v, b4v]).reshape(H, 1).astype(np.float32)

    gabs_np = np.stack([np.abs(gk) for gk in g], 1).astype(np.float32)
    bes_np = np.stack(be, 1).astype(np.float32)

    shared = {
        "w0": np.ascontiguousarray(W0f16),
        "w1t": np.ascontiguousarray(W1ts),
        "w1b": np.ascontiguousarray(W1bs),
        "w2": np.ascontiguousarray(W2f),
        "w3": np.ascontiguousarray(W3f),
        "w4l": np.ascontiguousarray(W4l.astype(FP16)),
        "w4r": np.ascontiguousarray(W4r.astype(FP16)),
        "c10": c1_0.reshape(H, 1),
        "gabs": gabs_np,
        "bes": bes_np,
        "b4d": b4dup,
    }
    in_maps = []
    for i in range(N_CORES):
        rows = pl[i * P_CORE : (i + 1) * P_CORE].reshape(R, C)
        xfm = np.ascontiguousarray(rows.T.astype(FP16))
        in_maps.append({"xfm": xfm, **shared})

    if _NC_CACHE is None:
        _NC_CACHE = build_nc()
    res = run_bass_kernel_spmd(_NC_CACHE, in_maps, list(range(N_CORES)))

    outs = []
    for i in range(N_CORES):
        o = np.asarray(res.results[i]["out"]).astype(np.float32)  # [128, R/2]
        # unpack: o[half*64+f, ci*1024+t] = y[f, ci*2048 + half*1024 + t]
        v = o.reshape(2, O, NCH, PS)            # [half, o, ci, t]
        y = v.transpose(1, 2, 0, 3).reshape(O, R)  # [o, ci, half, t] -> [o, R]
        outs.append(y.T.reshape(P_CORE, T, O))
    full = np.concatenate(outs, 0)
    return full.reshape(B, A, T, O)
